# revision 51
# baseline (speedup 1.0000x reference)
"""Trainium2 Bass kernel for nn_AnchorModule (8-core SPMD, data-parallel over batch).

Device layout conventions (per core, see build_program):
  selection rows r = 81*frame + anchor ; tiles t hold rows [128t, 128t+128)
  pixel (r, k) lives at activation free index 1024*(t//4) + 8*p + k, lane = t%4
  activations are channel-major: partition = 32*lane + ch (or 48/64-wide pairs)
"""
import math
import numpy as np
import ml_dtypes
from contextlib import ExitStack

from concourse import bass, bacc, mybir
from concourse.tile import TileContext

F32 = mybir.dt.float32
F32R = mybir.dt.float32r
BF16 = mybir.dt.bfloat16
U32 = mybir.dt.uint32
AF = mybir.ActivationFunctionType
OP = mybir.AluOpType
AX = mybir.AxisListType

S = 81
NS = 8
ZS, YS, XS = 9, 3, 3
GAP = 209
BF = ml_dtypes.bfloat16


class Cfg:
    def __init__(self, cores=8, FPC=64, N=1024, L=32, FEAT=28):
        self.cores, self.FPC, self.N, self.L, self.FEAT = cores, FPC, N, L, FEAT
        self.B = cores * FPC // L
        self.R = FPC * S
        self.NT = math.ceil(self.R / 128)
        self.TG = math.ceil(self.NT / 4)
        self.FREE = self.TG * 1024
        self.VL = min(32, FPC)
        self.FB = FPC // self.VL
        self.NXH = max(1, math.ceil(FPC * N * 4 / 131072))
        assert FPC % self.NXH == 0
        self.FPH = FPC // self.NXH
        self.NPIX = self.R * NS * cores
        self.NV4 = FPC * (ZS - 2) * cores
        self.NV5 = FPC * (ZS - 6) * cores
        self.NV6 = FPC * cores


def bn_scale_bias(nc, pool, pp, sums_cc, gb, npix, lrep, C, CR, eps_t=None):
    """sums_cc [C,2] (sum,sumsq) allreduced -> scale/bias replicated [CR,2]."""
    inv = 1.0 / npix
    mom = pool.tile([C, 2], F32, tag="bn_mom")
    nc.vector.tensor_scalar(mom[:], sums_cc[:], inv, None, op0=OP.mult)
    var = pool.tile([C, 1], F32, tag="bn_var")
    m2 = pool.tile([C, 1], F32, tag="bn_m2")
    nc.vector.tensor_tensor(out=m2[:], in0=mom[:, 0:1], in1=mom[:, 0:1], op=OP.mult)
    nc.vector.tensor_tensor(out=var[:], in0=mom[:, 1:2], in1=m2[:], op=OP.subtract)
    nc.vector.tensor_scalar(var[:], var[:], 0.0, None, op0=OP.max)
    sd = pool.tile([C, 1], F32, tag="bn_sd")
    nc.scalar.activation(sd[:], var[:], AF.Sqrt, bias=eps_t[0:C, 0:1])
    rs = pool.tile([C, 1], F32, tag="bn_rs")
    nc.vector.reciprocal(rs[:], sd[:])
    scbi = pool.tile([C, 2], F32, tag="bn_scbi")
    nc.vector.tensor_tensor(out=scbi[:, 0:1], in0=rs[:], in1=gb[:, 0:1], op=OP.mult)
    t = pool.tile([C, 1], F32, tag="bn_t")
    nc.vector.tensor_tensor(out=t[:], in0=mom[:, 0:1], in1=scbi[:, 0:1], op=OP.mult)
    nc.vector.tensor_tensor(out=scbi[:, 1:2], in0=gb[:, 1:2], in1=t[:], op=OP.subtract)
    reps = pp.tile([128, 1024], F32, tag="ps")
    nc.tensor.matmul(reps[0:CR, 0:2], lhsT=lrep[:], rhs=scbi[:],
                     start=True, stop=True)
    out = pool.tile([CR, 2], F32, tag="bn_out")
    nc.scalar.activation(out[:], reps[0:CR, 0:2], AF.Copy)
    return out


def build_program(cfg: Cfg):
    nc = bass.Bass(target_bir_lowering=False)
    FPC, N, VL, FB = cfg.FPC, cfg.N, cfg.VL, cfg.FB
    R, NT, TG, FREE = cfg.R, cfg.NT, cfg.TG, cfg.FREE
    L, B, cores = cfg.L, cfg.B, cfg.cores
    NXH, FPH = cfg.NXH, cfg.FPH

    dp = lambda n, s, d: nc.declare_dram_parameter(n, s, d, isOutput=False)
    xt = dp("xt", [FPC * N, 32], BF16)
    xyzdr = dp("xyzdr", [3, FPC * N], F32)
    xyzr = dp("xyzr", [3 * VL, FB * N], F32)
    a2dw = dp("a2dw", [4, GAP * FPC + 128], F32)
    wsel = dp("wsel", [3 * VL, VL], F32)
    wd3 = dp("wd3", [3, 32], F32)
    wtil = dp("wtil", [128, 64], BF16)
    w2 = dp("w2", [128, 96], BF16)
    w3 = dp("w3", [96, 128], BF16)
    watA = dp("watA", [128, 4], BF16)
    watB = dp("watB", [128, 4], BF16)
    brA = dp("brA", [4, 128], BF16)
    brB = dp("brB", [4, 128], BF16)
    ab4 = dp("ab4", [4, 1], F32)
    gbs = {k: dp(f"gb{k}", [c, 2], F32) for k, c in
           [(1, 32), (2, 48), (3, 64), (4, 96), (5, 128), (6, 64)]}
    ls32 = dp("ls32", [128, 32], F32)
    ls48 = dp("ls48", [96, 48], F32)
    ls64 = dp("ls64", [128, 64], F32)
    lr32 = dp("lr32", [32, 128], F32)
    lr48 = dp("lr48", [48, 96], F32)
    lr64 = dp("lr64", [64, 128], F32)
    vw1 = dp("vw1", [64, 27 * 96], BF16)
    vw2 = dp("vw2", [96, 5 * 128], BF16)
    vw3 = dp("vw3", [128, 3 * 64], BF16)
    wihT = dp("wihT", [64, 3 * 256], F32)
    whhT = dp("whhT", [64, 3 * 256], F32)
    lb12 = dp("lb12", [64, 24], F32)
    h0T = dp("h0T", [64, 3 * B], F32)
    c0T = dp("c0T", [64, 3 * B], F32)
    foff = dp("foff", [128, NT], U32)
    pm48a = dp("pm48a", [96, 1024], BF16)
    pm48b = dp("pm48b", [96, 1024], BF16)
    identb = dp("identb", [128, 128], BF16)

    attn_o = nc.declare_dram_parameter("attn_o", [4, FREE], BF16, isOutput=True)
    avec_o = nc.declare_dram_parameter("avec_o", [64, L * B], F32, isOutput=True)
    hncn_o = nc.declare_dram_parameter("hncn_o", [64, 6 * B], F32, isOutput=True)

    shr = dict(addr_space="Shared") if cores > 4 else {}
    ccd = {}
    for k, c in [(1, 32), (2, 48), (3, 64), (4, 96), (5, 128), (6, 64)]:
        ccd[k] = (nc.dram_tensor(f"cci{k}", [c, 2], F32),
                  nc.dram_tensor(f"cco{k}", [c, 2], F32, **shr))
    ag_in = nc.dram_tensor("ag_in", [64, FPC], F32)
    ag_out = nc.dram_tensor("ag_out", [cores * 64, FPC], F32, **shr)
    groups = [list(range(cores))]

    with TileContext(nc) as tc, ExitStack() as ctx:
        cpool = ctx.enter_context(tc.tile_pool(name="consts", bufs=1))
        big = ctx.enter_context(tc.tile_pool(name="big", bufs=1))
        work = ctx.enter_context(tc.tile_pool(name="work", bufs=3))
        gp = ctx.enter_context(tc.tile_pool(name="gp", bufs=20))
        pp = ctx.enter_context(tc.tile_pool(name="pp", bufs=4, space="PSUM"))

        def load(t):
            tile = cpool.tile(list(t.shape), t.dtype, name=t.name + "_sb",
                              tag=t.name + "_sb")
            nc.gpsimd.dma_start(out=tile[:], in_=t[:, :])
            return tile

        wsel_sb = load(wsel)
        wd3_sb = load(wd3)
        wtil_sb = load(wtil)
        w2_sb = load(w2)
        w3_sb = load(w3)
        watA_sb, watB_sb = load(watA), load(watB)
        brA_sb, brB_sb = load(brA), load(brB)
        ab4_sb = load(ab4)
        gb_sb = {k: load(v) for k, v in gbs.items()}
        ls32_sb, ls48_sb, ls64_sb = load(ls32), load(ls48), load(ls64)
        lr32_sb, lr48_sb, lr64_sb = load(lr32), load(lr48), load(lr64)
        vw1_sb, vw2_sb, vw3_sb = load(vw1), load(vw2), load(vw3)
        wihT_sb, whhT_sb = load(wihT), load(whhT)
        lb12_sb = load(lb12)
        h0_sb, c0_sb = load(h0T), load(c0T)
        foff_sb = load(foff)
        pm48a_sb, pm48b_sb = load(pm48a), load(pm48b)

        ident = load(identb)
        scr = big.tile([128, min(512, FREE)], F32, tag="scr")
        scrbf = scr[:].bitcast(BF16)
        eps_t = cpool.tile([128, 1], F32)
        nc.vector.memset(eps_t[:], 1e-5)

        # ---------------- P1: distance prep ----------------
        pn_sb = work.tile([VL, FB * N], F32, tag="pn", bufs=1)
        for i in range(math.ceil(FB * N / 512)):
            w_ = min(512, FB * N - 512 * i)
            xrc = work.tile([3 * VL, 512], F32, tag="xrc", bufs=1)
            nc.gpsimd.dma_start(out=xrc[:, 0:w_], in_=xyzr[:, 512 * i:512 * i + w_])
            nc.vector.tensor_tensor(out=xrc[:, 0:w_], in0=xrc[:, 0:w_],
                                    in1=xrc[:, 0:w_], op=OP.mult)
            pnps = pp.tile([128, 1024], F32, tag="ps")
            nc.tensor.matmul(pnps[0:VL, 0:w_], lhsT=wsel_sb[:],
                             rhs=xrc[:, 0:w_], start=True, stop=True)
            nc.scalar.activation(pn_sb[:, 512 * i:512 * i + w_], pnps[0:VL, 0:w_],
                                 AF.Copy)
        # ---------------- T1R ----------------
        t1r = cpool.tile([128, TG * 128], BF16)
        for T in range(TG):
            t1ps = pp.tile([128, 1024], F32, tag="ps")
            for lane in range(4):
                r0 = 512 * T + 128 * lane
                sl = t1ps[32 * lane:32 * (lane + 1), 0:128]
                if r0 >= R:
                    nc.vector.memset(sl, 0.0)
                    continue
                col, r = 0, r0
                while r < min(r0 + 128, R):
                    f = r // S
                    s0 = r - f * S
                    n = min(S - s0, r0 + 128 - r)
                    t1w = work.tile([3, 128], F32, tag="t1w", bufs=2)
                    nc.gpsimd.dma_start(
                        out=t1w[:, 0:n],
                        in_=a2dw[0:3, GAP * f + s0:GAP * f + s0 + n])
                    nc.tensor.matmul(t1ps[32 * lane:32 * (lane + 1), col:col + n],
                                     lhsT=wd3_sb[:], rhs=t1w[:, 0:n], start=True,
                                     stop=True, tile_position=(0, 32 * lane))
                    col += n
                    r += n
                if col < 128:
                    nc.vector.memset(t1ps[32 * lane:32 * (lane + 1), col:128], 0.0)
            nc.scalar.activation(t1r[:, 128 * T:128 * (T + 1)], t1ps[:, 0:128], AF.Copy)

        # ---------------- P2: selection ----------------
        idxbuf = cpool.tile([128, NT * 8], U32)
        for t in range(NT):
            scps = pp.tile([128, 1024], F32, tag="ps")
            r0, r1 = 128 * t, min(128 * (t + 1), R)
            f0, f1 = r0 // S, (r1 - 1) // S
            rhts = {}
            lhts = {}
            for f in range(f0, f1 + 1):
                rh = work.tile([4, N], F32, tag="rht", bufs=3)
                nc.gpsimd.dma_start(out=rh[0:3, :], in_=xyzdr[:, N * f:N * (f + 1)])
                v, Fb = f % VL, f // VL
                pps_ = pn_sb[:].ap[0][0]
                psrc = bass.AP(pn_sb.tensor, pn_sb[:].offset + v * pps_ + N * Fb,
                               [[pps_, 1], [1, N]])
                nc.gpsimd.dma_start(out=rh[3:4, :], in_=psrc)
                rhts[f] = rh
                lh = work.tile([4, 128], F32, tag="lht", bufs=3)
                nc.gpsimd.dma_start(out=lh[:], in_=a2dw[:, 128 * (t + f):128 * (t + f + 1)])
                lhts[f] = lh
            for half in range(math.ceil(N / 512)):
                w_ = min(512, N - 512 * half)
                for j, f in enumerate(range(f0, f1 + 1)):
                    nc.tensor.matmul(scps[:, 512 * half:512 * half + w_],
                                     lhsT=lhts[f][:],
                                     rhs=rhts[f][:, 512 * half:512 * half + w_],
                                     start=(j == 0), stop=(f == f1))
            vals = work.tile([128, 8], F32, tag="vals", bufs=2)
            nc.vector.max(out=vals[:], in_=scps[:, 0:N])
            idx8 = work.tile([128, 8], U32, tag="idx8", bufs=2)
            nc.vector.max_index(out=idx8[:], in_max=vals[:], in_values=scps[:, 0:N])
            nc.vector.tensor_tensor(out=idxbuf[:, 8 * t:8 * (t + 1)], in0=idx8[:],
                                    in1=foff_sb[:, t:t + 1].to_broadcast([128, 8]),
                                    op=OP.add)

        # ---------------- P3: gather + transpose -> XRAW ----------------
        xraw = big.tile([128, FREE], BF16, tag="s1")
        xrps = xraw[:].ap[0][0]
        for T in range(TG):
            tiles = list(range(4 * T, min(4 * T + 4, NT)))
            gt = {}
            for t in tiles:
                for k in range(NS):
                    g = gp.tile([128, 32], BF16, tag="gath")
                    nc.gpsimd.indirect_dma_start(
                        out=g[:], out_offset=None, in_=xt[:, :],
                        in_offset=bass.IndirectOffsetOnAxis(
                            ap=idxbuf[:, 8 * t + k:8 * t + k + 1], axis=0))
                    gt[(t, k)] = g
            for kh in range(2):
                xps = pp.tile([128, 2048], BF16, tag="ps")
                for t in tiles:
                    lane = t % 4
                    for kq in range(4):
                        k = 4 * kh + kq
                        nc.tensor.transpose(
                            xps[32 * lane:32 * (lane + 1), 128 * kq:128 * (kq + 1)],
                            in_=gt[(t, k)][:], identity=ident[:],
                            tile_position=(0, 32 * lane))
                np_ = 32 * len(tiles)
                dst = bass.AP(xraw.tensor, xraw[:].offset + 1024 * T + 4 * kh,
                              [[xrps, np_], [1, 4], [8, 128]])
                nc.scalar.activation(dst, xps[0:np_, 0:512], AF.Copy)
        lastT = TG - 1
        nlanes = NT - 4 * lastT

        def pquads(p0, p1):
            out = []
            while p0 < p1:
                if p0 == 0:
                    n = p1
                elif p0 in (32, 96):
                    n = min(32, p1 - p0)
                elif p0 == 64:
                    n = min(64, p1 - p0)
                else:
                    n = min(32 - p0 % 32, p1 - p0)
                out.append((p0, p0 + n))
                p0 += n
            return out

        if nlanes < 4:
            for q0, q1 in pquads(32 * nlanes, 128):
                nc.vector.memset(xraw[q0:q1, 1024 * lastT:FREE], 0.0)
        padr = NT * 128 - R
        if padr > 0:
            lane = (NT - 1) % 4
            p0 = 128 - padr
            padap = bass.AP(xraw.tensor,
                            xraw[:].offset + 32 * lane * xrps + 1024 * lastT + 8 * p0,
                            [[xrps, 32], [8, padr], [1, 8]])
            nc.vector.memset(padap, 0.0)

        # ---------------- P4: y1 + h1 ----------------
        h1 = big.tile([128, FREE], BF16, tag="s2")
        h1ps_ = h1[:].ap[0][0]
        t1ps_ = t1r[:].ap[0][0]
        for i in range(FREE // 512):
            y1ps = pp.tile([128, 1024], F32, tag="ps")
            sl = slice(512 * i, 512 * (i + 1))
            nc.tensor.matmul(y1ps[0:64, 0:512], lhsT=wtil_sb[0:64, :],
                             rhs=xraw[0:64, sl], start=True, stop=True)
            nc.tensor.matmul(y1ps[64:128, 0:512], lhsT=wtil_sb[64:128, :],
                             rhs=xraw[64:128, sl], start=True, stop=True,
                             tile_position=(64, 64))
            T, kh = i // 2, i % 2
            t1ap = bass.AP(t1r.tensor, t1r[:].offset + 128 * T + 64 * kh,
                           [[t1ps_, 128], [1, 64], [0, 8]])
            nc.vector.tensor_tensor(out=h1[:, sl], in0=y1ps[:, 0:512], in1=t1ap,
                                    op=OP.add)

        # ---------------- BN1 ----------------
        scr = big.tile([128, min(512, FREE)], F32, tag="scr")
        scrbf = scr[:].bitcast(BF16)

        SCW = min(512, FREE)

        def layer_stats2(h, C, tagp):
            nsq = math.ceil(FREE / SCW)
            sm = work.tile([C, 2 * nsq], F32, tag=tagp, bufs=1)
            for i in range(nsq):
                w_ = min(SCW, FREE - SCW * i)
                nc.vector.tensor_scalar(scrbf[0:C, 0:w_], h[0:C, SCW * i:SCW * i + w_],
                                        1.0, None, op0=OP.mult, op1=OP.add,
                                        accum_out=sm[:, i:i + 1])
                nc.scalar.activation(scr[0:C, 0:w_], h[0:C, SCW * i:SCW * i + w_],
                                     AF.Square, accum_out=sm[:, nsq + i:nsq + i + 1])
            tot = work.tile([C, 2], F32, tag=tagp + "t")
            if nsq > 1:
                nc.vector.tensor_reduce(tot[:, 0:1], sm[:, 0:nsq], axis=AX.X, op=OP.add)
                nc.vector.tensor_reduce(tot[:, 1:2], sm[:, nsq:2 * nsq], axis=AX.X, op=OP.add)
            else:
                nc.vector.tensor_copy(tot[:], sm[:])
            return tot

        def bn_roundtrip(k, lane_sums, C, CR, lrep, npix):
            cci, cco = ccd[k]
            st = work.tile([C, 2], F32, tag=f"st{k}")
            nc.scalar.activation(st[:], lane_sums, AF.Copy)
            nc.sync.dma_start(out=cci[:, :], in_=st[:])
            nc.gpsimd.collective_compute("AllReduce", OP.add, replica_groups=groups,
                                         ins=[cci.ap().opt()], outs=[cco.ap().opt()])
            stg = work.tile([C, 2], F32, tag=f"stg{k}")
            nc.sync.dma_start(out=stg[:], in_=cco[:, :])
            return bn_scale_bias(nc, work, pp, stg, gb_sb[k], npix, lrep, C, CR, eps_t)

        def zero_pads(h, chw, lanes_in):
            hps2 = h[:].ap[0][0]
            for li, lane in enumerate(lanes_in):
                if lane >= nlanes:
                    nc.vector.memset(h[chw * li:chw * (li + 1), 1024 * lastT:FREE], 0.0)
            lp = (NT - 1) % 4
            if padr > 0 and lp in lanes_in and lp < nlanes:
                li = lanes_in.index(lp)
                p0 = 128 - padr
                ap_ = bass.AP(h.tensor,
                              h[:].offset + chw * li * hps2 + 1024 * lastT + 8 * p0,
                              [[hps2, chw], [8, padr], [1, 8]])
                nc.vector.memset(ap_, 0.0)

        t1_ = layer_stats2(h1, 128, "bn1")
        lc = pp.tile([128, 1024], F32, tag="ps")
        nc.tensor.matmul(lc[0:32, 0:2], lhsT=ls32_sb[:], rhs=t1_[:],
                         start=True, stop=True)
        sb1 = bn_roundtrip(1, lc[0:32, 0:2], 32, 128, lr32_sb, cfg.NPIX)
        nc.scalar.activation(h1[:], h1[:], AF.Relu,
                             scale=sb1[:, 0:1], bias=sb1[:, 1:2])
        zero_pads(h1, 32, [0, 1, 2, 3])

        # ---------------- L2 ----------------
        h2a = big.tile([96, FREE], BF16, tag="s1", name="h2a")
        h2b = big.tile([96, FREE], BF16, tag="s3", name="h2b")
        for h2x, rb in [(h2a, 0), (h2b, 64)]:
            for i in range(FREE // 512):
                l2ps = pp.tile([128, 1024], F32, tag="ps")
                sl = slice(512 * i, 512 * (i + 1))
                nc.tensor.matmul(l2ps[0:96, 0:512], lhsT=w2_sb[rb:rb + 64, :],
                                 rhs=h1[rb:rb + 64, sl], start=True, stop=True,
                                 tile_position=(rb, 0))
                nc.scalar.activation(h2x[:, sl], l2ps[0:96, 0:512], AF.Copy)
        ta = layer_stats2(h2a, 96, "bn2a")
        tb = layer_stats2(h2b, 96, "bn2b")
        tab = work.tile([96, 2], F32, tag="bn2ab")
        nc.vector.tensor_tensor(out=tab[:], in0=ta[:], in1=tb[:], op=OP.add)
        lc2 = pp.tile([128, 1024], F32, tag="ps")
        nc.tensor.matmul(lc2[0:48, 0:2], lhsT=ls48_sb[:], rhs=tab[:],
                         start=True, stop=True)
        sb2 = bn_roundtrip(2, lc2[0:48, 0:2], 48, 96, lr48_sb, cfg.NPIX)
        nc.scalar.activation(h2a[:], h2a[:], AF.Relu, scale=sb2[:, 0:1], bias=sb2[:, 1:2])
        nc.scalar.activation(h2b[:], h2b[:], AF.Relu, scale=sb2[:, 0:1], bias=sb2[:, 1:2])
        for h2x, pm in [(h2a, pm48a_sb), (h2b, pm48b_sb)]:
            reg = slice(1024 * lastT, 1024 * lastT + 1024)
            nc.vector.tensor_tensor(out=h2x[:, reg], in0=h2x[:, reg], in1=pm[:],
                                    op=OP.mult)

        # ---------------- L3 ----------------
        h3a = big.tile([128, FREE], BF16, tag="s2", name="h3a")
        h3b = big.tile([128, FREE], BF16, tag="s4", name="h3b")
        for h3x, h2x in [(h3a, h2a), (h3b, h2b)]:
            for i in range(FREE // 512):
                l3ps = pp.tile([128, 1024], F32, tag="ps")
                sl = slice(512 * i, 512 * (i + 1))
                nc.tensor.matmul(l3ps[:, 0:512], lhsT=w3_sb[:],
                                 rhs=h2x[:, sl], start=True, stop=True)
                nc.scalar.activation(h3x[:, sl], l3ps[:, 0:512], AF.Copy)
        ta3 = layer_stats2(h3a, 128, "bn3a")
        tb3 = layer_stats2(h3b, 128, "bn3b")
        lc3 = pp.tile([128, 1024], F32, tag="ps")
        nc.tensor.matmul(lc3[0:64, 0:2], lhsT=ls64_sb[:], rhs=ta3[:],
                         start=True, stop=False)
        nc.tensor.matmul(lc3[0:64, 0:2], lhsT=ls64_sb[:], rhs=tb3[:],
                         start=False, stop=True)
        sb3 = bn_roundtrip(3, lc3[0:64, 0:2], 64, 128, lr64_sb, cfg.NPIX)
        nc.scalar.activation(h3a[:], h3a[:], AF.Relu, scale=sb3[:, 0:1], bias=sb3[:, 1:2])
        nc.scalar.activation(h3b[:], h3b[:], AF.Relu, scale=sb3[:, 0:1], bias=sb3[:, 1:2])
        zero_pads(h3a, 64, [0, 1])
        zero_pads(h3b, 64, [2, 3])

        # ---------------- attention ----------------
        el = cpool.tile([4, FREE], BF16)  # becomes attn in place
        for i in range(FREE // 1024):
            elps = pp.tile([128, 1024], F32, tag="ps")
            for half in range(2):
                sl = slice(1024 * i + 512 * half, 1024 * i + 512 * (half + 1))
                psl = slice(512 * half, 512 * (half + 1))
                nc.tensor.matmul(elps[0:4, psl], lhsT=watA_sb[:], rhs=h3a[:, sl],
                                 start=True, stop=False)
                nc.tensor.matmul(elps[0:4, psl], lhsT=watB_sb[:], rhs=h3b[:, sl],
                                 start=False, stop=True)
            nc.scalar.activation(el[:, 1024 * i:1024 * (i + 1)], elps[0:4, :],
                                 AF.Exp, bias=ab4_sb[:, 0:1])
        elps_ = el[:].ap[0][0]
        den = work.tile([4, FREE // 8], F32, tag="den", bufs=1)
        el3 = bass.AP(el.tensor, el[:].offset, [[elps_, 4], [8, FREE // 8], [1, 8]])
        nc.vector.tensor_reduce(den[:], el3, axis=AX.X, op=OP.add)
        nc.vector.reciprocal(den[:], den[:])
        recb = work.tile([4, FREE // 8], BF16, tag="recb", bufs=1)
        nc.vector.tensor_copy(recb[:], den[:])
        attn = el
        rb3 = bass.AP(recb.tensor, recb[:].offset,
                      [[recb[:].ap[0][0], 4], [1, FREE // 8], [0, 8]])
        nc.vector.tensor_tensor(out=el3, in0=el3, in1=rb3, op=OP.mult)
        nc.sync.dma_start(out=attn_o[:, :], in_=attn[:])

        # pooled vec = sum_k h3 * attn64
        vecs = []
        for h3x, br in [(h3a, brA_sb), (h3b, brB_sb)]:
            at64 = big.tile([128, FREE], BF16, tag="s1", name="at64")
            for i in range(FREE // 512):
                bps = pp.tile([128, 1024], F32, tag="ps")
                sl = slice(512 * i, 512 * (i + 1))
                nc.tensor.matmul(bps[:, 0:512], lhsT=br[:], rhs=attn[:, sl],
                                 start=True, stop=True)
                nc.scalar.activation(at64[:, sl], bps[:, 0:512], AF.Copy)
            wv = big.tile([128, FREE], BF16, tag="s3", name="wv")
            nc.vector.tensor_tensor(out=wv[:], in0=h3x[:], in1=at64[:], op=OP.mult)
            wv3 = bass.AP(wv.tensor, wv[:].offset,
                          [[wv[:].ap[0][0], 128], [8, FREE // 8], [1, 8]])
            vecb = work.tile([128, FREE // 8], BF16, tag="vecb", bufs=1)
            with nc.allow_low_precision("pooled vec fits bf16"):
                nc.vector.tensor_reduce(vecb[:], wv3, axis=AX.X, op=OP.add)
            vecs.append(vecb)

        vcm = cpool.tile([64, 512 * TG], BF16)
        for pi, vecb in enumerate(vecs):
            for li in range(2):
                lane = 2 * pi + li
                src = bass.AP(vecb.tensor, vecb[:].offset + 64 * li * vecb[:].ap[0][0],
                              [[vecb[:].ap[0][0], 64], [128, TG], [1, 128]])
                dst = bass.AP(vcm.tensor, vcm[:].offset + 128 * lane,
                              [[vcm[:].ap[0][0], 64], [512, TG], [1, 128]])
                # clip columns beyond R for the last T-group
                nc.sync.dma_start(out=dst, in_=src)

        # ---------------- VoxelNet ----------------
        Z1 = ZS - 2
        v1ps = pp.tile([128, 1024], F32, tag="ps")
        vps = vcm[:].ap[0][0]
        for i in range(27):
            dz, y, x = i // 9, (i // 3) % 3, i % 3
            s0 = dz * 9 + y * 3 + x
            rhs = bass.AP(vcm.tensor, vcm[:].offset + s0,
                          [[vps, 64], [S, FPC], [9, Z1]])
            nc.tensor.matmul(v1ps[0:96, 0:FPC * Z1], lhsT=vw1_sb[:, 96 * i:96 * (i + 1)],
                             rhs=rhs, start=(i == 0), stop=(i == 26))

        # direct scale/bias without replication for voxel layers
        def vox_sb(k, ps_ap, C, nloc, nglob):
            sm = work.tile([C, 2], F32, tag=f"vx{k}")
            vb = work.tile([C, nloc], BF16, tag=f"vxb{k}")
            nc.vector.tensor_scalar(vb[:], ps_ap, 1.0, None, op0=OP.mult,
                                    op1=OP.add, accum_out=sm[:, 0:1])
            nc.scalar.activation(scr[0:C, 0:nloc], ps_ap, AF.Square,
                                 accum_out=sm[:, 1:2])
            cci, cco = ccd[k]
            nc.sync.dma_start(out=cci[:, :], in_=sm[:])
            nc.gpsimd.collective_compute("AllReduce", OP.add, replica_groups=groups,
                                         ins=[cci.ap().opt()], outs=[cco.ap().opt()])
            stg = work.tile([C, 2], F32, tag=f"vxg{k}")
            nc.sync.dma_start(out=stg[:], in_=cco[:, :])
            inv = 1.0 / nglob
            mom = work.tile([C, 2], F32, tag=f"vmom{k}")
            nc.vector.tensor_scalar(mom[:], stg[:], inv, None, op0=OP.mult)
            var = work.tile([C, 1], F32, tag=f"vvar{k}")
            m2 = work.tile([C, 1], F32, tag=f"vm2{k}")
            nc.vector.tensor_tensor(out=m2[:], in0=mom[:, 0:1], in1=mom[:, 0:1], op=OP.mult)
            nc.vector.tensor_tensor(out=var[:], in0=mom[:, 1:2], in1=m2[:], op=OP.subtract)
            nc.vector.tensor_scalar(var[:], var[:], 0.0, None, op0=OP.max)
            sd = work.tile([C, 1], F32, tag=f"vsd{k}")
            nc.scalar.activation(sd[:], var[:], AF.Sqrt, bias=eps_t[0:C, 0:1])
            rs = work.tile([C, 1], F32, tag=f"vrs{k}")
            nc.vector.reciprocal(rs[:], sd[:])
            scbi = work.tile([C, 2], F32, tag=f"vscbi{k}")
            nc.vector.tensor_tensor(out=scbi[:, 0:1], in0=rs[:], in1=gb_sb[k][:, 0:1], op=OP.mult)
            tt = work.tile([C, 1], F32, tag=f"vt{k}")
            nc.vector.tensor_tensor(out=tt[:], in0=mom[:, 0:1], in1=scbi[:, 0:1], op=OP.mult)
            nc.vector.tensor_tensor(out=scbi[:, 1:2], in0=gb_sb[k][:, 1:2], in1=tt[:], op=OP.subtract)
            return scbi

        sb4 = vox_sb(4, v1ps[0:96, 0:FPC * Z1], 96, FPC * Z1, cfg.NV4)
        v1 = cpool.tile([96, FPC * Z1], BF16)
        nc.scalar.activation(v1[:], v1ps[0:96, 0:FPC * Z1], AF.Relu,
                             scale=sb4[:, 0:1], bias=sb4[:, 1:2])

        Z2 = ZS - 6
        v2ps = pp.tile([128, 1024], F32, tag="ps")
        v1s = v1[:].ap[0][0]
        for i in range(5):
            rhs = bass.AP(v1.tensor, v1[:].offset + i,
                          [[v1s, 96], [Z1, FPC], [1, Z2]])
            nc.tensor.matmul(v2ps[:, 0:FPC * Z2], lhsT=vw2_sb[:, 128 * i:128 * (i + 1)],
                             rhs=rhs, start=(i == 0), stop=(i == 4))
        sb5 = vox_sb(5, v2ps[:, 0:FPC * Z2], 128, FPC * Z2, cfg.NV5)
        v2 = cpool.tile([128, FPC * Z2], BF16)
        nc.scalar.activation(v2[:], v2ps[:, 0:FPC * Z2], AF.Relu,
                             scale=sb5[:, 0:1], bias=sb5[:, 1:2])

        v3ps = pp.tile([128, 1024], F32, tag="ps")
        v2s = v2[:].ap[0][0]
        for i in range(3):
            rhs = bass.AP(v2.tensor, v2[:].offset + i, [[v2s, 128], [Z2, FPC]])
            nc.tensor.matmul(v3ps[0:64, 0:FPC], lhsT=vw3_sb[:, 64 * i:64 * (i + 1)],
                             rhs=rhs, start=(i == 0), stop=(i == 2))
        sb6 = vox_sb(6, v3ps[0:64, 0:FPC], 64, FPC, cfg.NV6)
        v3 = cpool.tile([64, FPC], F32)
        nc.scalar.activation(v3[:], v3ps[0:64, 0:FPC], AF.Relu,
                             scale=sb6[:, 0:1], bias=sb6[:, 1:2])

        # ---------------- AllGather + LSTM ----------------
        nc.sync.dma_start(out=ag_in[:, :], in_=v3[:])
        nc.gpsimd.collective_compute("AllGather", OP.bypass, replica_groups=groups,
                                     ins=[ag_in.ap().opt()], outs=[ag_out.ap().opt()])
        seq = cpool.tile([64, L * B], F32)
        QF = FPC // L
        sps = seq[:].ap[0][0]
        for c in range(cores):
            for q in range(QF):
                src_ap = bass.AP(ag_out, 64 * FPC * c + L * q,
                                 [[FPC, 64], [1, L]])
                dst_ap = bass.AP(seq.tensor, seq[:].offset + QF * c + q,
                                 [[sps, 64], [B, L]])
                nc.sync.dma_start(out=dst_ap, in_=src_ap)

        lbg = cpool.tile([64, 12], F32)
        nc.vector.tensor_tensor(out=lbg[:], in0=lb12_sb[:, 0:12],
                                in1=lb12_sb[:, 12:24], op=OP.add)

        avec = cpool.tile([64, L * B], F32)
        hbuf = [cpool.tile([64, B], F32, name=f'hbuf{i}') for i in range(2)]
        cbuf = [cpool.tile([64, B], F32, name=f'cbuf{i}') for i in range(3)]
        sig = work
        for t in range(L):
            for l in range(3):
                xsrc = (seq[:, B * t:B * (t + 1)] if l == 0 else hbuf[l - 1][:])
                hsrc = (h0_sb[:, B * l:B * (l + 1)] if t == 0 else
                        (hbuf[l][:] if l < 2 else avec[:, B * (t - 1):B * t]))
                csrc = c0_sb[:, B * l:B * (l + 1)] if t == 0 else cbuf[l][:]
                psA = pp.tile([128, 1024], F32, tag="ps")
                psB = pp.tile([128, 1024], F32, tag="ps")
                # gate g at psX[0:64, 512*half : 512*half+B]: (i,f)->A, (g,o)->B
                for gi, (ps_, half) in enumerate(
                        [(psA, 0), (psA, 1), (psB, 0), (psB, 1)]):
                    o = 512 * half
                    wsl = slice(256 * l + 64 * gi, 256 * l + 64 * (gi + 1))
                    nc.tensor.matmul(ps_[0:64, o:o + B],
                                     lhsT=wihT_sb[:, wsl],
                                     rhs=xsrc, start=True, stop=False)
                    nc.tensor.matmul(ps_[0:64, o:o + B],
                                     lhsT=whhT_sb[:, wsl],
                                     rhs=hsrc, start=False, stop=True)
                si = work.tile([64, B], F32, tag="si")
                nc.scalar.activation(si[:], psA[0:64, 0:B], AF.Sigmoid,
                                     bias=lbg[:, 4 * l:4 * l + 1])
                sf = work.tile([64, B], F32, tag="sf")
                nc.scalar.activation(sf[:], psA[0:64, 512:512 + B], AF.Sigmoid,
                                     bias=lbg[:, 4 * l + 1:4 * l + 2])
                tg = work.tile([64, B], F32, tag="tg")
                nc.scalar.activation(tg[:], psB[0:64, 0:B], AF.Tanh,
                                     bias=lbg[:, 4 * l + 2:4 * l + 3])
                so = work.tile([64, B], F32, tag="so")
                nc.scalar.activation(so[:], psB[0:64, 512:512 + B], AF.Sigmoid,
                                     bias=lbg[:, 4 * l + 3:4 * l + 4])
                t1 = work.tile([64, B], F32, tag="lt1")
                nc.vector.tensor_tensor(out=t1[:], in0=si[:], in1=tg[:], op=OP.mult)
                t2 = work.tile([64, B], F32, tag="lt2")
                nc.vector.tensor_tensor(out=t2[:], in0=sf[:], in1=csrc, op=OP.mult)
                nc.vector.tensor_tensor(out=cbuf[l][:], in0=t1[:], in1=t2[:], op=OP.add)
                tc_ = work.tile([64, B], F32, tag="ltc")
                nc.scalar.activation(tc_[:], cbuf[l][:], AF.Tanh)
                hdst = hbuf[l][:] if l < 2 else avec[:, B * t:B * (t + 1)]
                nc.vector.tensor_tensor(out=hdst, in0=so[:], in1=tc_[:], op=OP.mult)
        nc.sync.dma_start(out=avec_o[:, :], in_=avec[:])
        hncn = cpool.tile([64, 6 * B], F32)
        for l in range(3):
            hs = hbuf[l][:] if l < 2 else avec[:, B * (L - 1):B * L]
            nc.vector.tensor_copy(hncn[:, B * l:B * (l + 1)], hs)
            nc.vector.tensor_copy(hncn[:, B * (3 + l):B * (4 + l)], cbuf[l][:])
        nc.sync.dma_start(out=hncn_o[:, :], in_=hncn[:])

    from concourse.bacc import _bass_rust as _br
    _br.move_matmul_waits_to_ldweights(nc.m)
    _br.generate_event_semaphores(nc)
    return nc


# --------------------------------------------------------------------------
# host side
# --------------------------------------------------------------------------

def anchor_template():
    ax = np.arange(XS, dtype=np.float32) * 0.3 - 0.3
    ay = np.arange(YS, dtype=np.float32) * 0.3 - 0.3
    az = np.arange(ZS, dtype=np.float32) * 0.3 - 0.3
    zz, yy, xx = np.meshgrid(az, ay, ax, indexing="ij")
    return np.stack([xx, yy, zz], -1).reshape(S, 3).astype(np.float32)


def blockdiag2(w):
    a, b = w.shape
    out = np.zeros((2 * a, 2 * b), w.dtype)
    out[:a, :b] = w
    out[a:, b:] = w
    return out


def prep_inputs(cfg: Cfg, inp):
    FPC, N, VL, FB, L, Bg = cfg.FPC, cfg.N, cfg.VL, cfg.FB, cfg.L, cfg.B
    R, NT = cfg.R, cfg.NT
    x = np.asarray(inp["x"], np.float32)
    g_loc = np.asarray(inp["g_loc"], np.float32).reshape(-1, 2)
    tpl = anchor_template()
    FEAT = cfg.FEAT

    pw1 = np.asarray(inp["pw1"], np.float32)
    wt = np.zeros((32, 32), np.float32)
    wt[0:3, :] = pw1[:, 0:3].T * 0 + pw1[:, 3:6].T
    wt[3:3 + FEAT - 3, :] = pw1[:, 6:3 + FEAT].T
    wtil_h = np.vstack([blockdiag2(wt), blockdiag2(wt)]).astype(BF)
    wd3_h = ((pw1[:, 0:3] - pw1[:, 3:6]).T / 2.0).astype(np.float32)
    w2_h = np.vstack([blockdiag2(np.asarray(inp["pw2"], np.float32).T)] * 2).astype(BF)
    w3_h = blockdiag2(np.asarray(inp["pw3"], np.float32).T).astype(BF)
    aw = np.asarray(inp["attn_w"], np.float32)[0]
    watA_h = np.zeros((128, 4), np.float32)
    watA_h[0:64, 0] = aw
    watA_h[64:128, 1] = aw
    watB_h = np.zeros((128, 4), np.float32)
    watB_h[0:64, 2] = aw
    watB_h[64:128, 3] = aw
    brA_h = np.zeros((4, 128), np.float32)
    brA_h[0, 0:64] = 1
    brA_h[1, 64:128] = 1
    brB_h = np.zeros((4, 128), np.float32)
    brB_h[2, 0:64] = 1
    brB_h[3, 64:128] = 1
    ab4_h = np.full((4, 1), np.float32(np.asarray(inp["attn_b"]).reshape(-1)[0]))

    gbs = {1: ("pg1", "pbt1"), 2: ("pg2", "pbt2"), 3: ("pg3", "pbt3"),
           4: ("vg1", "vbt1"), 5: ("vg2", "vbt2"), 6: ("vg3", "vbt3")}
    gb_h = {k: np.stack([np.asarray(inp[g], np.float32),
                         np.asarray(inp[b], np.float32)], -1)
            for k, (g, b) in gbs.items()}

    ls32_h = np.zeros((128, 32), np.float32)
    for lane in range(4):
        ls32_h[32 * lane + np.arange(32), np.arange(32)] = 1
    ls48_h = np.zeros((96, 48), np.float32)
    for a in range(2):
        ls48_h[48 * a + np.arange(48), np.arange(48)] = 1
    ls64_h = np.zeros((128, 64), np.float32)
    for a in range(2):
        ls64_h[64 * a + np.arange(64), np.arange(64)] = 1
    lr32_h = np.zeros((32, 128), np.float32)
    for lane in range(4):
        lr32_h[np.arange(32), 32 * lane + np.arange(32)] = 1
    lr48_h = np.zeros((48, 96), np.float32)
    for a in range(2):
        lr48_h[np.arange(48), 48 * a + np.arange(48)] = 1
    lr64_h = np.zeros((64, 128), np.float32)
    for a in range(2):
        lr64_h[np.arange(64), 64 * a + np.arange(64)] = 1

    vw1_h = np.asarray(inp["vw1"], np.float32).transpose(1, 2, 3, 4, 0).reshape(64, 27 * 96).astype(BF)
    vw2_h = np.asarray(inp["vw2"], np.float32).transpose(1, 2, 0, 3, 4).reshape(96, 5 * 128).astype(BF)
    vw3_h = np.asarray(inp["vw3"], np.float32).transpose(1, 2, 0, 3, 4).reshape(128, 3 * 64).astype(BF)

    wih = np.asarray(inp["lstm_wih"], np.float32)
    whh = np.asarray(inp["lstm_whh"], np.float32)
    wihT_h = wih.transpose(0, 2, 1).transpose(1, 0, 2).reshape(64, 3 * 256).astype(np.float32)
    whhT_h = whh.transpose(0, 2, 1).transpose(1, 0, 2).reshape(64, 3 * 256).astype(np.float32)
    bih = np.asarray(inp["lstm_bih"], np.float32)
    bhh = np.asarray(inp["lstm_bhh"], np.float32)
    lb12_h = np.zeros((64, 24), np.float32)
    for l in range(3):
        for g in range(4):
            lb12_h[:, 4 * l + g] = bih[l, 64 * g:64 * (g + 1)]
            lb12_h[:, 12 + 4 * l + g] = bhh[l, 64 * g:64 * (g + 1)]
    h0T_h = np.asarray(inp["h0"], np.float32).transpose(0, 2, 1).transpose(1, 0, 2).reshape(64, 3 * Bg)
    c0T_h = np.asarray(inp["c0"], np.float32).transpose(0, 2, 1).transpose(1, 0, 2).reshape(64, 3 * Bg)

    lastT = cfg.TG - 1
    nlanes = NT - 4 * lastT

    def padmask(chw, lanes_in):
        pm = np.zeros((len(lanes_in) * chw, 1024), np.float32)
        for li, lane in enumerate(lanes_in):
            t = 4 * lastT + lane
            if t >= NT:
                continue
            for prow in range(128):
                if 128 * t + prow < R:
                    pm[chw * li:chw * (li + 1), 8 * prow:8 * (prow + 1)] = 1
        return pm.astype(BF)

    pm48a_h = padmask(48, [0, 1])
    pm48b_h = padmask(48, [2, 3])

    foff_h = np.zeros((128, NT), np.uint32)
    rr = (128 * np.arange(NT)[None, :] + np.arange(128)[:, None])
    valid = rr < R
    foff_h[valid] = (rr[valid] // S).astype(np.uint32) * N

    t4_h = np.zeros((4, GAP * FPC + 128), np.float32)
    cols = (GAP * np.arange(FPC)[:, None] + np.arange(S)[None, :]).ravel()
    for r in range(3):
        t4_h[r, cols] = np.tile(2.0 * tpl[:, r], FPC)
    t4_h[3, cols] = -1.0

    wsel_h = np.zeros((3 * VL, VL), np.float32)
    for v in range(VL):
        wsel_h[3 * v + np.arange(3), v] = 1

    in_maps = []
    for c in range(cfg.cores):
        xc = x[c * FPC:(c + 1) * FPC]  # [FPC, N, FEAT]
        xt_h = np.zeros((FPC * N, 32), BF)
        xt_h[:, :FEAT] = xc.reshape(FPC * N, FEAT).astype(BF)
        xyz = xc[:, :, 0:3].reshape(FB, VL, N, 3)
        xyzr_h = np.zeros((3 * VL, FB * N), np.float32)
        for r in range(3):
            m = xyz[:, :, :, r].transpose(1, 0, 2).reshape(VL, FB * N)
            xyzr_h[3 * np.arange(VL) + r] = m
        a4 = t4_h.copy()
        gseg = g_loc[c * FPC:(c + 1) * FPC]
        for r in range(2):
            a4[r, cols] += np.repeat(2.0 * gseg[:, r], S)
        a2dw_h = a4.astype(np.float32)
        xyzdr_h = np.ascontiguousarray(
            xc[:, :, 0:3].reshape(-1, 3).T.astype(np.float32))
        in_maps.append({
            "xt": xt_h, "xyzdr": xyzdr_h, "xyzr": xyzr_h, "a2dw": a2dw_h,
            "wsel": wsel_h, "wd3": wd3_h, "wtil": wtil_h,
            "w2": w2_h, "w3": w3_h, "watA": watA_h.astype(BF),
            "watB": watB_h.astype(BF), "brA": brA_h.astype(BF),
            "brB": brB_h.astype(BF), "ab4": ab4_h,
            **{f"gb{k}": v for k, v in gb_h.items()},
            "ls32": ls32_h, "ls48": ls48_h, "ls64": ls64_h,
            "lr32": lr32_h, "lr48": lr48_h, "lr64": lr64_h,
            "vw1": vw1_h, "vw2": vw2_h, "vw3": vw3_h,
            "wihT": wihT_h, "whhT": whhT_h, "lb12": lb12_h,
            "h0T": h0T_h, "c0T": c0T_h, "foff": foff_h,
            "pm48a": pm48a_h, "pm48b": pm48b_h,
            "identb": np.eye(128, dtype=np.float32).astype(BF),
        })
    return in_maps


def postprocess(cfg: Cfg, results):
    R, NT, TG, L, Bg = cfg.R, cfg.NT, cfg.TG, cfg.L, cfg.B
    attn_full = np.zeros((cfg.cores * R, NS), np.float32)
    for c, res in enumerate(results):
        arr = np.asarray(res["attn_o"]).astype(np.float32).reshape(4, TG, 128, NS)
        for t in range(NT):
            lane, T = t % 4, t // 4
            r0 = 128 * t
            n = min(128, R - r0)
            attn_full[c * R + r0:c * R + r0 + n] = arr[lane, T, :n]
    res0 = results[0]
    avec = np.asarray(res0["avec_o"]).reshape(64, L, Bg)
    a_vec = avec.transpose(2, 1, 0).astype(np.float32)
    hncn = np.asarray(res0["hncn_o"]).reshape(64, 6, Bg)
    hn = hncn[:, 0:3].transpose(1, 2, 0).astype(np.float32)
    cn = hncn[:, 3:6].transpose(1, 2, 0).astype(np.float32)
    return (a_vec, attn_full[:, :, None].astype(np.float32), hn, cn)


_CACHE = {}


def kernel(**inputs):
    cfg = Cfg()
    if "nc" not in _CACHE:
        _CACHE["nc"] = build_program(cfg)
    nc = _CACHE["nc"]
    in_maps = prep_inputs(cfg, inputs)
    from concourse import bass_utils
    res = bass_utils.run_bass_kernel_spmd(nc, in_maps, core_ids=list(range(cfg.cores)))
    return postprocess(cfg, res.results)


# revision 52
# speedup vs baseline: 1.0349x; 1.0349x over previous
"""Trainium2 Bass kernel for nn_AnchorModule (8-core SPMD, data-parallel over batch).

Device layout conventions (per core, see build_program):
  selection rows r = 81*frame + anchor ; tiles t hold rows [128t, 128t+128)
  pixel (r, k) lives at activation free index 1024*(t//4) + 8*p + k, lane = t%4
  activations are channel-major: partition = 32*lane + ch (or 48/64-wide pairs)
"""
import math
import numpy as np
import ml_dtypes
from contextlib import ExitStack

from concourse import bass, bacc, mybir
from concourse.tile import TileContext

F32 = mybir.dt.float32
F32R = mybir.dt.float32r
BF16 = mybir.dt.bfloat16
U32 = mybir.dt.uint32
AF = mybir.ActivationFunctionType
OP = mybir.AluOpType
AX = mybir.AxisListType

S = 81
NS = 8
ZS, YS, XS = 9, 3, 3
GAP = 209
BF = ml_dtypes.bfloat16


class Cfg:
    def __init__(self, cores=8, FPC=64, N=1024, L=32, FEAT=28):
        self.cores, self.FPC, self.N, self.L, self.FEAT = cores, FPC, N, L, FEAT
        self.B = cores * FPC // L
        self.R = FPC * S
        self.NT = math.ceil(self.R / 128)
        self.TG = math.ceil(self.NT / 4)
        self.FREE = self.TG * 1024
        self.VL = min(32, FPC)
        self.FB = FPC // self.VL
        self.NXH = max(1, math.ceil(FPC * N * 4 / 131072))
        assert FPC % self.NXH == 0
        self.FPH = FPC // self.NXH
        self.NPIX = self.R * NS * cores
        self.NV4 = FPC * (ZS - 2) * cores
        self.NV5 = FPC * (ZS - 6) * cores
        self.NV6 = FPC * cores


def bn_scale_bias(nc, pool, pp, sums_cc, gb, npix, lrep, C, CR, eps_t=None):
    """sums_cc [C,2] (sum,sumsq) allreduced -> scale/bias replicated [CR,2]."""
    inv = 1.0 / npix
    mom = pool.tile([C, 2], F32, tag="bn_mom")
    nc.vector.tensor_scalar(mom[:], sums_cc[:], inv, None, op0=OP.mult)
    var = pool.tile([C, 1], F32, tag="bn_var")
    m2 = pool.tile([C, 1], F32, tag="bn_m2")
    nc.vector.tensor_tensor(out=m2[:], in0=mom[:, 0:1], in1=mom[:, 0:1], op=OP.mult)
    nc.vector.tensor_tensor(out=var[:], in0=mom[:, 1:2], in1=m2[:], op=OP.subtract)
    nc.vector.tensor_scalar(var[:], var[:], 0.0, None, op0=OP.max)
    sd = pool.tile([C, 1], F32, tag="bn_sd")
    nc.scalar.activation(sd[:], var[:], AF.Sqrt, bias=eps_t[0:C, 0:1])
    rs = pool.tile([C, 1], F32, tag="bn_rs")
    nc.vector.reciprocal(rs[:], sd[:])
    scbi = pool.tile([C, 2], F32, tag="bn_scbi")
    nc.vector.tensor_tensor(out=scbi[:, 0:1], in0=rs[:], in1=gb[:, 0:1], op=OP.mult)
    t = pool.tile([C, 1], F32, tag="bn_t")
    nc.vector.tensor_tensor(out=t[:], in0=mom[:, 0:1], in1=scbi[:, 0:1], op=OP.mult)
    nc.vector.tensor_tensor(out=scbi[:, 1:2], in0=gb[:, 1:2], in1=t[:], op=OP.subtract)
    reps = pp.tile([128, 1024], F32, tag="ps")
    nc.tensor.matmul(reps[0:CR, 0:2], lhsT=lrep[:], rhs=scbi[:],
                     start=True, stop=True)
    out = pool.tile([CR, 2], F32, tag="bn_out")
    nc.scalar.activation(out[:], reps[0:CR, 0:2], AF.Copy)
    return out


def build_program(cfg: Cfg):
    nc = bass.Bass(target_bir_lowering=False)
    FPC, N, VL, FB = cfg.FPC, cfg.N, cfg.VL, cfg.FB
    R, NT, TG, FREE = cfg.R, cfg.NT, cfg.TG, cfg.FREE
    L, B, cores = cfg.L, cfg.B, cfg.cores
    NXH, FPH = cfg.NXH, cfg.FPH

    dp = lambda n, s, d: nc.declare_dram_parameter(n, s, d, isOutput=False)
    xt = dp("xt", [FPC * N, 32], BF16)
    xyzdr = dp("xyzdr", [3, FPC * N], F32)
    xyzr = dp("xyzr", [3 * VL, FB * N], F32)
    a2dw = dp("a2dw", [4, GAP * FPC + 128], F32)
    wsel = dp("wsel", [3 * VL, VL], F32)
    wd3 = dp("wd3", [3, 32], F32)
    wtil = dp("wtil", [128, 64], BF16)
    w2 = dp("w2", [128, 96], BF16)
    w3 = dp("w3", [96, 128], BF16)
    watA = dp("watA", [128, 4], BF16)
    watB = dp("watB", [128, 4], BF16)
    brA = dp("brA", [4, 128], BF16)
    brB = dp("brB", [4, 128], BF16)
    ab4 = dp("ab4", [4, 1], F32)
    gbs = {k: dp(f"gb{k}", [c, 2], F32) for k, c in
           [(1, 32), (2, 48), (3, 64), (4, 96), (5, 128), (6, 64)]}
    ls32 = dp("ls32", [128, 32], F32)
    ls48 = dp("ls48", [96, 48], F32)
    ls64 = dp("ls64", [128, 64], F32)
    lr32 = dp("lr32", [32, 128], F32)
    lr48 = dp("lr48", [48, 96], F32)
    lr64 = dp("lr64", [64, 128], F32)
    vw1 = dp("vw1", [64, 27 * 96], BF16)
    vw2 = dp("vw2", [96, 5 * 128], BF16)
    vw3 = dp("vw3", [128, 3 * 64], BF16)
    wihT = dp("wihT", [64, 3 * 256], F32)
    whhT = dp("whhT", [64, 3 * 256], F32)
    lb12 = dp("lb12", [64, 24], F32)
    h0T = dp("h0T", [64, 3 * B], F32)
    c0T = dp("c0T", [64, 3 * B], F32)
    foff = dp("foff", [128, NT], U32)
    pm48a = dp("pm48a", [96, 1024], BF16)
    pm48b = dp("pm48b", [96, 1024], BF16)
    identb = dp("identb", [128, 128], BF16)

    attn_o = nc.declare_dram_parameter("attn_o", [4, FREE], BF16, isOutput=True)
    avec_o = nc.declare_dram_parameter("avec_o", [64, L * B], F32, isOutput=True)
    hncn_o = nc.declare_dram_parameter("hncn_o", [64, 6 * B], F32, isOutput=True)

    shr = dict(addr_space="Shared") if cores > 4 else {}
    ccd = {}
    for k, c in [(1, 32), (2, 48), (3, 64), (4, 96), (5, 128), (6, 64)]:
        ccd[k] = (nc.dram_tensor(f"cci{k}", [c, 2], F32),
                  nc.dram_tensor(f"cco{k}", [c, 2], F32, **shr))
    ag_in = nc.dram_tensor("ag_in", [64, FPC], F32)
    ag_out = nc.dram_tensor("ag_out", [cores * 64, FPC], F32, **shr)
    groups = [list(range(cores))]

    with TileContext(nc) as tc, ExitStack() as ctx:
        cpool = ctx.enter_context(tc.tile_pool(name="consts", bufs=1))
        big = ctx.enter_context(tc.tile_pool(name="big", bufs=1))
        work = ctx.enter_context(tc.tile_pool(name="work", bufs=3))
        gp = ctx.enter_context(tc.tile_pool(name="gp", bufs=20))
        pp = ctx.enter_context(tc.tile_pool(name="pp", bufs=4, space="PSUM"))

        def load(t):
            tile = cpool.tile(list(t.shape), t.dtype, name=t.name + "_sb",
                              tag=t.name + "_sb")
            nc.sync.dma_start(out=tile[:], in_=t[:, :])
            return tile

        wsel_sb = load(wsel)
        wd3_sb = load(wd3)
        wtil_sb = load(wtil)
        w2_sb = load(w2)
        w3_sb = load(w3)
        watA_sb, watB_sb = load(watA), load(watB)
        brA_sb, brB_sb = load(brA), load(brB)
        ab4_sb = load(ab4)
        gb_sb = {k: load(v) for k, v in gbs.items()}
        ls32_sb, ls48_sb, ls64_sb = load(ls32), load(ls48), load(ls64)
        lr32_sb, lr48_sb, lr64_sb = load(lr32), load(lr48), load(lr64)
        vw1_sb, vw2_sb, vw3_sb = load(vw1), load(vw2), load(vw3)
        wihT_sb, whhT_sb = load(wihT), load(whhT)
        lb12_sb = load(lb12)
        h0_sb, c0_sb = load(h0T), load(c0T)
        foff_sb = load(foff)
        pm48a_sb, pm48b_sb = load(pm48a), load(pm48b)

        ident = load(identb)
        scr = big.tile([128, min(512, FREE)], F32, tag="scr")
        scrbf = scr[:].bitcast(BF16)
        eps_t = cpool.tile([128, 1], F32)
        nc.vector.memset(eps_t[:], 1e-5)

        # ---------------- P1: distance prep ----------------
        pn_sb = work.tile([VL, FB * N], F32, tag="pn", bufs=1)
        for i in range(math.ceil(FB * N / 512)):
            w_ = min(512, FB * N - 512 * i)
            xrc = work.tile([3 * VL, 512], F32, tag="xrc", bufs=1)
            nc.sync.dma_start(out=xrc[:, 0:w_], in_=xyzr[:, 512 * i:512 * i + w_])
            nc.vector.tensor_tensor(out=xrc[:, 0:w_], in0=xrc[:, 0:w_],
                                    in1=xrc[:, 0:w_], op=OP.mult)
            pnps = pp.tile([128, 1024], F32, tag="ps")
            nc.tensor.matmul(pnps[0:VL, 0:w_], lhsT=wsel_sb[:],
                             rhs=xrc[:, 0:w_], start=True, stop=True)
            nc.scalar.activation(pn_sb[:, 512 * i:512 * i + w_], pnps[0:VL, 0:w_],
                                 AF.Copy)
        # ---------------- T1R ----------------
        t1r = cpool.tile([128, TG * 128], BF16)
        for T in range(TG):
            t1ps = pp.tile([128, 1024], F32, tag="ps")
            for lane in range(4):
                r0 = 512 * T + 128 * lane
                sl = t1ps[32 * lane:32 * (lane + 1), 0:128]
                if r0 >= R:
                    nc.vector.memset(sl, 0.0)
                    continue
                col, r = 0, r0
                while r < min(r0 + 128, R):
                    f = r // S
                    s0 = r - f * S
                    n = min(S - s0, r0 + 128 - r)
                    t1w = work.tile([3, 128], F32, tag="t1w", bufs=2)
                    nc.sync.dma_start(
                        out=t1w[:, 0:n],
                        in_=a2dw[0:3, GAP * f + s0:GAP * f + s0 + n])
                    nc.tensor.matmul(t1ps[32 * lane:32 * (lane + 1), col:col + n],
                                     lhsT=wd3_sb[:], rhs=t1w[:, 0:n], start=True,
                                     stop=True, tile_position=(0, 32 * lane))
                    col += n
                    r += n
                if col < 128:
                    nc.vector.memset(t1ps[32 * lane:32 * (lane + 1), col:128], 0.0)
            nc.scalar.activation(t1r[:, 128 * T:128 * (T + 1)], t1ps[:, 0:128], AF.Copy)

        # ---------------- P2: selection ----------------
        idxbuf = cpool.tile([128, NT * 8], U32)
        for t in range(NT):
            scps = pp.tile([128, 1024], F32, tag="ps")
            r0, r1 = 128 * t, min(128 * (t + 1), R)
            f0, f1 = r0 // S, (r1 - 1) // S
            rhts = {}
            lhts = {}
            for f in range(f0, f1 + 1):
                rh = work.tile([4, N], F32, tag="rht", bufs=3)
                nc.sync.dma_start(out=rh[0:3, :], in_=xyzdr[:, N * f:N * (f + 1)])
                v, Fb = f % VL, f // VL
                pps_ = pn_sb[:].ap[0][0]
                psrc = bass.AP(pn_sb.tensor, pn_sb[:].offset + v * pps_ + N * Fb,
                               [[pps_, 1], [1, N]])
                nc.sync.dma_start(out=rh[3:4, :], in_=psrc)
                rhts[f] = rh
                lh = work.tile([4, 128], F32, tag="lht", bufs=3)
                nc.sync.dma_start(out=lh[:], in_=a2dw[:, 128 * (t + f):128 * (t + f + 1)])
                lhts[f] = lh
            for half in range(math.ceil(N / 512)):
                w_ = min(512, N - 512 * half)
                for j, f in enumerate(range(f0, f1 + 1)):
                    nc.tensor.matmul(scps[:, 512 * half:512 * half + w_],
                                     lhsT=lhts[f][:],
                                     rhs=rhts[f][:, 512 * half:512 * half + w_],
                                     start=(j == 0), stop=(f == f1))
            vals = work.tile([128, 8], F32, tag="vals", bufs=2)
            nc.vector.max(out=vals[:], in_=scps[:, 0:N])
            idx8 = work.tile([128, 8], U32, tag="idx8", bufs=2)
            nc.vector.max_index(out=idx8[:], in_max=vals[:], in_values=scps[:, 0:N])
            nc.vector.tensor_tensor(out=idxbuf[:, 8 * t:8 * (t + 1)], in0=idx8[:],
                                    in1=foff_sb[:, t:t + 1].to_broadcast([128, 8]),
                                    op=OP.add)

        # ---------------- P3: gather + transpose -> XRAW ----------------
        xraw = big.tile([128, FREE], BF16, tag="s1")
        xrps = xraw[:].ap[0][0]
        for T in range(TG):
            tiles = list(range(4 * T, min(4 * T + 4, NT)))
            gt = {}
            for t in tiles:
                for k in range(NS):
                    g = gp.tile([128, 32], BF16, tag="gath")
                    nc.gpsimd.indirect_dma_start(
                        out=g[:], out_offset=None, in_=xt[:, :],
                        in_offset=bass.IndirectOffsetOnAxis(
                            ap=idxbuf[:, 8 * t + k:8 * t + k + 1], axis=0))
                    gt[(t, k)] = g
            for kh in range(2):
                xps = pp.tile([128, 2048], BF16, tag="ps")
                for t in tiles:
                    lane = t % 4
                    for kq in range(4):
                        k = 4 * kh + kq
                        nc.tensor.transpose(
                            xps[32 * lane:32 * (lane + 1), 128 * kq:128 * (kq + 1)],
                            in_=gt[(t, k)][:], identity=ident[:],
                            tile_position=(0, 32 * lane))
                np_ = 32 * len(tiles)
                dst = bass.AP(xraw.tensor, xraw[:].offset + 1024 * T + 4 * kh,
                              [[xrps, np_], [1, 4], [8, 128]])
                nc.scalar.activation(dst, xps[0:np_, 0:512], AF.Copy)
        lastT = TG - 1
        nlanes = NT - 4 * lastT

        def pquads(p0, p1):
            out = []
            while p0 < p1:
                if p0 == 0:
                    n = p1
                elif p0 in (32, 96):
                    n = min(32, p1 - p0)
                elif p0 == 64:
                    n = min(64, p1 - p0)
                else:
                    n = min(32 - p0 % 32, p1 - p0)
                out.append((p0, p0 + n))
                p0 += n
            return out

        if nlanes < 4:
            for q0, q1 in pquads(32 * nlanes, 128):
                nc.vector.memset(xraw[q0:q1, 1024 * lastT:FREE], 0.0)
        padr = NT * 128 - R
        if padr > 0:
            lane = (NT - 1) % 4
            p0 = 128 - padr
            padap = bass.AP(xraw.tensor,
                            xraw[:].offset + 32 * lane * xrps + 1024 * lastT + 8 * p0,
                            [[xrps, 32], [8, padr], [1, 8]])
            nc.vector.memset(padap, 0.0)

        # ---------------- P4: y1 + h1 ----------------
        h1 = big.tile([128, FREE], BF16, tag="s2")
        h1ps_ = h1[:].ap[0][0]
        t1ps_ = t1r[:].ap[0][0]
        for i in range(FREE // 512):
            y1ps = pp.tile([128, 1024], F32, tag="ps")
            sl = slice(512 * i, 512 * (i + 1))
            nc.tensor.matmul(y1ps[0:64, 0:512], lhsT=wtil_sb[0:64, :],
                             rhs=xraw[0:64, sl], start=True, stop=True)
            nc.tensor.matmul(y1ps[64:128, 0:512], lhsT=wtil_sb[64:128, :],
                             rhs=xraw[64:128, sl], start=True, stop=True,
                             tile_position=(64, 64))
            T, kh = i // 2, i % 2
            t1ap = bass.AP(t1r.tensor, t1r[:].offset + 128 * T + 64 * kh,
                           [[t1ps_, 128], [1, 64], [0, 8]])
            nc.vector.tensor_tensor(out=h1[:, sl], in0=y1ps[:, 0:512], in1=t1ap,
                                    op=OP.add)

        # ---------------- BN1 ----------------
        scr = big.tile([128, min(512, FREE)], F32, tag="scr")
        scrbf = scr[:].bitcast(BF16)

        SCW = min(512, FREE)

        def layer_stats2(h, C, tagp):
            nsq = math.ceil(FREE / SCW)
            sm = work.tile([C, 2 * nsq], F32, tag=tagp, bufs=1)
            for i in range(nsq):
                w_ = min(SCW, FREE - SCW * i)
                nc.vector.tensor_scalar(scrbf[0:C, 0:w_], h[0:C, SCW * i:SCW * i + w_],
                                        1.0, None, op0=OP.mult, op1=OP.add,
                                        accum_out=sm[:, i:i + 1])
                nc.scalar.activation(scr[0:C, 0:w_], h[0:C, SCW * i:SCW * i + w_],
                                     AF.Square, accum_out=sm[:, nsq + i:nsq + i + 1])
            tot = work.tile([C, 2], F32, tag=tagp + "t")
            if nsq > 1:
                nc.vector.tensor_reduce(tot[:, 0:1], sm[:, 0:nsq], axis=AX.X, op=OP.add)
                nc.vector.tensor_reduce(tot[:, 1:2], sm[:, nsq:2 * nsq], axis=AX.X, op=OP.add)
            else:
                nc.vector.tensor_copy(tot[:], sm[:])
            return tot

        def bn_roundtrip(k, lane_sums, C, CR, lrep, npix):
            cci, cco = ccd[k]
            st = work.tile([C, 2], F32, tag=f"st{k}")
            nc.scalar.activation(st[:], lane_sums, AF.Copy)
            nc.sync.dma_start(out=cci[:, :], in_=st[:])
            nc.gpsimd.collective_compute("AllReduce", OP.add, replica_groups=groups,
                                         ins=[cci.ap().opt()], outs=[cco.ap().opt()])
            stg = work.tile([C, 2], F32, tag=f"stg{k}")
            nc.sync.dma_start(out=stg[:], in_=cco[:, :])
            return bn_scale_bias(nc, work, pp, stg, gb_sb[k], npix, lrep, C, CR, eps_t)

        def zero_pads(h, chw, lanes_in):
            hps2 = h[:].ap[0][0]
            for li, lane in enumerate(lanes_in):
                if lane >= nlanes:
                    nc.vector.memset(h[chw * li:chw * (li + 1), 1024 * lastT:FREE], 0.0)
            lp = (NT - 1) % 4
            if padr > 0 and lp in lanes_in and lp < nlanes:
                li = lanes_in.index(lp)
                p0 = 128 - padr
                ap_ = bass.AP(h.tensor,
                              h[:].offset + chw * li * hps2 + 1024 * lastT + 8 * p0,
                              [[hps2, chw], [8, padr], [1, 8]])
                nc.vector.memset(ap_, 0.0)

        t1_ = layer_stats2(h1, 128, "bn1")
        lc = pp.tile([128, 1024], F32, tag="ps")
        nc.tensor.matmul(lc[0:32, 0:2], lhsT=ls32_sb[:], rhs=t1_[:],
                         start=True, stop=True)
        sb1 = bn_roundtrip(1, lc[0:32, 0:2], 32, 128, lr32_sb, cfg.NPIX)
        nc.scalar.activation(h1[:], h1[:], AF.Relu,
                             scale=sb1[:, 0:1], bias=sb1[:, 1:2])
        zero_pads(h1, 32, [0, 1, 2, 3])

        # ---------------- L2 ----------------
        h2a = big.tile([96, FREE], BF16, tag="s1", name="h2a")
        h2b = big.tile([96, FREE], BF16, tag="s3", name="h2b")
        for h2x, rb in [(h2a, 0), (h2b, 64)]:
            for i in range(FREE // 512):
                l2ps = pp.tile([128, 1024], F32, tag="ps")
                sl = slice(512 * i, 512 * (i + 1))
                nc.tensor.matmul(l2ps[0:96, 0:512], lhsT=w2_sb[rb:rb + 64, :],
                                 rhs=h1[rb:rb + 64, sl], start=True, stop=True,
                                 tile_position=(rb, 0))
                nc.scalar.activation(h2x[:, sl], l2ps[0:96, 0:512], AF.Copy)
        ta = layer_stats2(h2a, 96, "bn2a")
        tb = layer_stats2(h2b, 96, "bn2b")
        tab = work.tile([96, 2], F32, tag="bn2ab")
        nc.vector.tensor_tensor(out=tab[:], in0=ta[:], in1=tb[:], op=OP.add)
        lc2 = pp.tile([128, 1024], F32, tag="ps")
        nc.tensor.matmul(lc2[0:48, 0:2], lhsT=ls48_sb[:], rhs=tab[:],
                         start=True, stop=True)
        sb2 = bn_roundtrip(2, lc2[0:48, 0:2], 48, 96, lr48_sb, cfg.NPIX)
        nc.scalar.activation(h2a[:], h2a[:], AF.Relu, scale=sb2[:, 0:1], bias=sb2[:, 1:2])
        nc.scalar.activation(h2b[:], h2b[:], AF.Relu, scale=sb2[:, 0:1], bias=sb2[:, 1:2])
        for h2x, pm in [(h2a, pm48a_sb), (h2b, pm48b_sb)]:
            reg = slice(1024 * lastT, 1024 * lastT + 1024)
            nc.vector.tensor_tensor(out=h2x[:, reg], in0=h2x[:, reg], in1=pm[:],
                                    op=OP.mult)

        # ---------------- L3 ----------------
        h3a = big.tile([128, FREE], BF16, tag="s2", name="h3a")
        h3b = big.tile([128, FREE], BF16, tag="s4", name="h3b")
        for h3x, h2x in [(h3a, h2a), (h3b, h2b)]:
            for i in range(FREE // 512):
                l3ps = pp.tile([128, 1024], F32, tag="ps")
                sl = slice(512 * i, 512 * (i + 1))
                nc.tensor.matmul(l3ps[:, 0:512], lhsT=w3_sb[:],
                                 rhs=h2x[:, sl], start=True, stop=True)
                nc.scalar.activation(h3x[:, sl], l3ps[:, 0:512], AF.Copy)
        ta3 = layer_stats2(h3a, 128, "bn3a")
        tb3 = layer_stats2(h3b, 128, "bn3b")
        lc3 = pp.tile([128, 1024], F32, tag="ps")
        nc.tensor.matmul(lc3[0:64, 0:2], lhsT=ls64_sb[:], rhs=ta3[:],
                         start=True, stop=False)
        nc.tensor.matmul(lc3[0:64, 0:2], lhsT=ls64_sb[:], rhs=tb3[:],
                         start=False, stop=True)
        sb3 = bn_roundtrip(3, lc3[0:64, 0:2], 64, 128, lr64_sb, cfg.NPIX)
        nc.scalar.activation(h3a[:], h3a[:], AF.Relu, scale=sb3[:, 0:1], bias=sb3[:, 1:2])
        nc.scalar.activation(h3b[:], h3b[:], AF.Relu, scale=sb3[:, 0:1], bias=sb3[:, 1:2])
        zero_pads(h3a, 64, [0, 1])
        zero_pads(h3b, 64, [2, 3])

        # ---------------- attention ----------------
        el = cpool.tile([4, FREE], BF16)  # becomes attn in place
        for i in range(FREE // 1024):
            elps = pp.tile([128, 1024], F32, tag="ps")
            for half in range(2):
                sl = slice(1024 * i + 512 * half, 1024 * i + 512 * (half + 1))
                psl = slice(512 * half, 512 * (half + 1))
                nc.tensor.matmul(elps[0:4, psl], lhsT=watA_sb[:], rhs=h3a[:, sl],
                                 start=True, stop=False)
                nc.tensor.matmul(elps[0:4, psl], lhsT=watB_sb[:], rhs=h3b[:, sl],
                                 start=False, stop=True)
            nc.scalar.activation(el[:, 1024 * i:1024 * (i + 1)], elps[0:4, :],
                                 AF.Exp, bias=ab4_sb[:, 0:1])
        elps_ = el[:].ap[0][0]
        den = work.tile([4, FREE // 8], F32, tag="den", bufs=1)
        el3 = bass.AP(el.tensor, el[:].offset, [[elps_, 4], [8, FREE // 8], [1, 8]])
        nc.vector.tensor_reduce(den[:], el3, axis=AX.X, op=OP.add)
        nc.vector.reciprocal(den[:], den[:])
        recb = work.tile([4, FREE // 8], BF16, tag="recb", bufs=1)
        nc.vector.tensor_copy(recb[:], den[:])
        attn = el
        rb3 = bass.AP(recb.tensor, recb[:].offset,
                      [[recb[:].ap[0][0], 4], [1, FREE // 8], [0, 8]])
        nc.vector.tensor_tensor(out=el3, in0=el3, in1=rb3, op=OP.mult)
        nc.sync.dma_start(out=attn_o[:, :], in_=attn[:])

        # pooled vec = sum_k h3 * attn64
        vecs = []
        for h3x, br in [(h3a, brA_sb), (h3b, brB_sb)]:
            at64 = big.tile([128, FREE], BF16, tag="s1", name="at64")
            for i in range(FREE // 512):
                bps = pp.tile([128, 1024], F32, tag="ps")
                sl = slice(512 * i, 512 * (i + 1))
                nc.tensor.matmul(bps[:, 0:512], lhsT=br[:], rhs=attn[:, sl],
                                 start=True, stop=True)
                nc.scalar.activation(at64[:, sl], bps[:, 0:512], AF.Copy)
            wv = big.tile([128, FREE], BF16, tag="s3", name="wv")
            nc.vector.tensor_tensor(out=wv[:], in0=h3x[:], in1=at64[:], op=OP.mult)
            wv3 = bass.AP(wv.tensor, wv[:].offset,
                          [[wv[:].ap[0][0], 128], [8, FREE // 8], [1, 8]])
            vecb = work.tile([128, FREE // 8], BF16, tag="vecb", bufs=1)
            with nc.allow_low_precision("pooled vec fits bf16"):
                nc.vector.tensor_reduce(vecb[:], wv3, axis=AX.X, op=OP.add)
            vecs.append(vecb)

        vcm = cpool.tile([64, 512 * TG], BF16)
        for pi, vecb in enumerate(vecs):
            for li in range(2):
                lane = 2 * pi + li
                src = bass.AP(vecb.tensor, vecb[:].offset + 64 * li * vecb[:].ap[0][0],
                              [[vecb[:].ap[0][0], 64], [128, TG], [1, 128]])
                dst = bass.AP(vcm.tensor, vcm[:].offset + 128 * lane,
                              [[vcm[:].ap[0][0], 64], [512, TG], [1, 128]])
                # clip columns beyond R for the last T-group
                nc.sync.dma_start(out=dst, in_=src)

        # ---------------- VoxelNet ----------------
        Z1 = ZS - 2
        v1ps = pp.tile([128, 1024], F32, tag="ps")
        vps = vcm[:].ap[0][0]
        for i in range(27):
            dz, y, x = i // 9, (i // 3) % 3, i % 3
            s0 = dz * 9 + y * 3 + x
            rhs = bass.AP(vcm.tensor, vcm[:].offset + s0,
                          [[vps, 64], [S, FPC], [9, Z1]])
            nc.tensor.matmul(v1ps[0:96, 0:FPC * Z1], lhsT=vw1_sb[:, 96 * i:96 * (i + 1)],
                             rhs=rhs, start=(i == 0), stop=(i == 26))

        # direct scale/bias without replication for voxel layers
        def vox_sb(k, ps_ap, C, nloc, nglob):
            sm = work.tile([C, 2], F32, tag=f"vx{k}")
            vb = work.tile([C, nloc], BF16, tag=f"vxb{k}")
            nc.vector.tensor_scalar(vb[:], ps_ap, 1.0, None, op0=OP.mult,
                                    op1=OP.add, accum_out=sm[:, 0:1])
            nc.scalar.activation(scr[0:C, 0:nloc], ps_ap, AF.Square,
                                 accum_out=sm[:, 1:2])
            cci, cco = ccd[k]
            nc.sync.dma_start(out=cci[:, :], in_=sm[:])
            nc.gpsimd.collective_compute("AllReduce", OP.add, replica_groups=groups,
                                         ins=[cci.ap().opt()], outs=[cco.ap().opt()])
            stg = work.tile([C, 2], F32, tag=f"vxg{k}")
            nc.sync.dma_start(out=stg[:], in_=cco[:, :])
            inv = 1.0 / nglob
            mom = work.tile([C, 2], F32, tag=f"vmom{k}")
            nc.vector.tensor_scalar(mom[:], stg[:], inv, None, op0=OP.mult)
            var = work.tile([C, 1], F32, tag=f"vvar{k}")
            m2 = work.tile([C, 1], F32, tag=f"vm2{k}")
            nc.vector.tensor_tensor(out=m2[:], in0=mom[:, 0:1], in1=mom[:, 0:1], op=OP.mult)
            nc.vector.tensor_tensor(out=var[:], in0=mom[:, 1:2], in1=m2[:], op=OP.subtract)
            nc.vector.tensor_scalar(var[:], var[:], 0.0, None, op0=OP.max)
            sd = work.tile([C, 1], F32, tag=f"vsd{k}")
            nc.scalar.activation(sd[:], var[:], AF.Sqrt, bias=eps_t[0:C, 0:1])
            rs = work.tile([C, 1], F32, tag=f"vrs{k}")
            nc.vector.reciprocal(rs[:], sd[:])
            scbi = work.tile([C, 2], F32, tag=f"vscbi{k}")
            nc.vector.tensor_tensor(out=scbi[:, 0:1], in0=rs[:], in1=gb_sb[k][:, 0:1], op=OP.mult)
            tt = work.tile([C, 1], F32, tag=f"vt{k}")
            nc.vector.tensor_tensor(out=tt[:], in0=mom[:, 0:1], in1=scbi[:, 0:1], op=OP.mult)
            nc.vector.tensor_tensor(out=scbi[:, 1:2], in0=gb_sb[k][:, 1:2], in1=tt[:], op=OP.subtract)
            return scbi

        sb4 = vox_sb(4, v1ps[0:96, 0:FPC * Z1], 96, FPC * Z1, cfg.NV4)
        v1 = cpool.tile([96, FPC * Z1], BF16)
        nc.scalar.activation(v1[:], v1ps[0:96, 0:FPC * Z1], AF.Relu,
                             scale=sb4[:, 0:1], bias=sb4[:, 1:2])

        Z2 = ZS - 6
        v2ps = pp.tile([128, 1024], F32, tag="ps")
        v1s = v1[:].ap[0][0]
        for i in range(5):
            rhs = bass.AP(v1.tensor, v1[:].offset + i,
                          [[v1s, 96], [Z1, FPC], [1, Z2]])
            nc.tensor.matmul(v2ps[:, 0:FPC * Z2], lhsT=vw2_sb[:, 128 * i:128 * (i + 1)],
                             rhs=rhs, start=(i == 0), stop=(i == 4))
        sb5 = vox_sb(5, v2ps[:, 0:FPC * Z2], 128, FPC * Z2, cfg.NV5)
        v2 = cpool.tile([128, FPC * Z2], BF16)
        nc.scalar.activation(v2[:], v2ps[:, 0:FPC * Z2], AF.Relu,
                             scale=sb5[:, 0:1], bias=sb5[:, 1:2])

        v3ps = pp.tile([128, 1024], F32, tag="ps")
        v2s = v2[:].ap[0][0]
        for i in range(3):
            rhs = bass.AP(v2.tensor, v2[:].offset + i, [[v2s, 128], [Z2, FPC]])
            nc.tensor.matmul(v3ps[0:64, 0:FPC], lhsT=vw3_sb[:, 64 * i:64 * (i + 1)],
                             rhs=rhs, start=(i == 0), stop=(i == 2))
        sb6 = vox_sb(6, v3ps[0:64, 0:FPC], 64, FPC, cfg.NV6)
        v3 = cpool.tile([64, FPC], F32)
        nc.scalar.activation(v3[:], v3ps[0:64, 0:FPC], AF.Relu,
                             scale=sb6[:, 0:1], bias=sb6[:, 1:2])

        # ---------------- AllGather + LSTM ----------------
        nc.sync.dma_start(out=ag_in[:, :], in_=v3[:])
        nc.gpsimd.collective_compute("AllGather", OP.bypass, replica_groups=groups,
                                     ins=[ag_in.ap().opt()], outs=[ag_out.ap().opt()])
        seq = cpool.tile([64, L * B], F32)
        QF = FPC // L
        sps = seq[:].ap[0][0]
        for c in range(cores):
            for q in range(QF):
                src_ap = bass.AP(ag_out, 64 * FPC * c + L * q,
                                 [[FPC, 64], [1, L]])
                dst_ap = bass.AP(seq.tensor, seq[:].offset + QF * c + q,
                                 [[sps, 64], [B, L]])
                nc.sync.dma_start(out=dst_ap, in_=src_ap)

        lbg = cpool.tile([64, 12], F32)
        nc.vector.tensor_tensor(out=lbg[:], in0=lb12_sb[:, 0:12],
                                in1=lb12_sb[:, 12:24], op=OP.add)

        avec = cpool.tile([64, L * B], F32)
        hbuf = [cpool.tile([64, B], F32, name=f'hbuf{i}') for i in range(2)]
        cbuf = [cpool.tile([64, B], F32, name=f'cbuf{i}') for i in range(3)]
        sig = work
        for t in range(L):
            for l in range(3):
                xsrc = (seq[:, B * t:B * (t + 1)] if l == 0 else hbuf[l - 1][:])
                hsrc = (h0_sb[:, B * l:B * (l + 1)] if t == 0 else
                        (hbuf[l][:] if l < 2 else avec[:, B * (t - 1):B * t]))
                csrc = c0_sb[:, B * l:B * (l + 1)] if t == 0 else cbuf[l][:]
                psA = pp.tile([128, 1024], F32, tag="ps")
                psB = pp.tile([128, 1024], F32, tag="ps")
                # gate g at psX[0:64, 512*half : 512*half+B]: (i,f)->A, (g,o)->B
                for gi, (ps_, half) in enumerate(
                        [(psA, 0), (psA, 1), (psB, 0), (psB, 1)]):
                    o = 512 * half
                    wsl = slice(256 * l + 64 * gi, 256 * l + 64 * (gi + 1))
                    nc.tensor.matmul(ps_[0:64, o:o + B],
                                     lhsT=wihT_sb[:, wsl],
                                     rhs=xsrc, start=True, stop=False)
                    nc.tensor.matmul(ps_[0:64, o:o + B],
                                     lhsT=whhT_sb[:, wsl],
                                     rhs=hsrc, start=False, stop=True)
                si = work.tile([64, B], F32, tag="si")
                nc.scalar.activation(si[:], psA[0:64, 0:B], AF.Sigmoid,
                                     bias=lbg[:, 4 * l:4 * l + 1])
                sf = work.tile([64, B], F32, tag="sf")
                nc.scalar.activation(sf[:], psA[0:64, 512:512 + B], AF.Sigmoid,
                                     bias=lbg[:, 4 * l + 1:4 * l + 2])
                tg = work.tile([64, B], F32, tag="tg")
                nc.scalar.activation(tg[:], psB[0:64, 0:B], AF.Tanh,
                                     bias=lbg[:, 4 * l + 2:4 * l + 3])
                so = work.tile([64, B], F32, tag="so")
                nc.scalar.activation(so[:], psB[0:64, 512:512 + B], AF.Sigmoid,
                                     bias=lbg[:, 4 * l + 3:4 * l + 4])
                t1 = work.tile([64, B], F32, tag="lt1")
                nc.vector.tensor_tensor(out=t1[:], in0=si[:], in1=tg[:], op=OP.mult)
                t2 = work.tile([64, B], F32, tag="lt2")
                nc.vector.tensor_tensor(out=t2[:], in0=sf[:], in1=csrc, op=OP.mult)
                nc.vector.tensor_tensor(out=cbuf[l][:], in0=t1[:], in1=t2[:], op=OP.add)
                tc_ = work.tile([64, B], F32, tag="ltc")
                nc.scalar.activation(tc_[:], cbuf[l][:], AF.Tanh)
                hdst = hbuf[l][:] if l < 2 else avec[:, B * t:B * (t + 1)]
                nc.vector.tensor_tensor(out=hdst, in0=so[:], in1=tc_[:], op=OP.mult)
        nc.sync.dma_start(out=avec_o[:, :], in_=avec[:])
        hncn = cpool.tile([64, 6 * B], F32)
        for l in range(3):
            hs = hbuf[l][:] if l < 2 else avec[:, B * (L - 1):B * L]
            nc.vector.tensor_copy(hncn[:, B * l:B * (l + 1)], hs)
            nc.vector.tensor_copy(hncn[:, B * (3 + l):B * (4 + l)], cbuf[l][:])
        nc.sync.dma_start(out=hncn_o[:, :], in_=hncn[:])

    from concourse.bacc import _bass_rust as _br
    _br.move_matmul_waits_to_ldweights(nc.m)
    _br.generate_event_semaphores(nc)
    return nc


# --------------------------------------------------------------------------
# host side
# --------------------------------------------------------------------------

def anchor_template():
    ax = np.arange(XS, dtype=np.float32) * 0.3 - 0.3
    ay = np.arange(YS, dtype=np.float32) * 0.3 - 0.3
    az = np.arange(ZS, dtype=np.float32) * 0.3 - 0.3
    zz, yy, xx = np.meshgrid(az, ay, ax, indexing="ij")
    return np.stack([xx, yy, zz], -1).reshape(S, 3).astype(np.float32)


def blockdiag2(w):
    a, b = w.shape
    out = np.zeros((2 * a, 2 * b), w.dtype)
    out[:a, :b] = w
    out[a:, b:] = w
    return out


def prep_inputs(cfg: Cfg, inp):
    FPC, N, VL, FB, L, Bg = cfg.FPC, cfg.N, cfg.VL, cfg.FB, cfg.L, cfg.B
    R, NT = cfg.R, cfg.NT
    x = np.asarray(inp["x"], np.float32)
    g_loc = np.asarray(inp["g_loc"], np.float32).reshape(-1, 2)
    tpl = anchor_template()
    FEAT = cfg.FEAT

    pw1 = np.asarray(inp["pw1"], np.float32)
    wt = np.zeros((32, 32), np.float32)
    wt[0:3, :] = pw1[:, 0:3].T * 0 + pw1[:, 3:6].T
    wt[3:3 + FEAT - 3, :] = pw1[:, 6:3 + FEAT].T
    wtil_h = np.vstack([blockdiag2(wt), blockdiag2(wt)]).astype(BF)
    wd3_h = ((pw1[:, 0:3] - pw1[:, 3:6]).T / 2.0).astype(np.float32)
    w2_h = np.vstack([blockdiag2(np.asarray(inp["pw2"], np.float32).T)] * 2).astype(BF)
    w3_h = blockdiag2(np.asarray(inp["pw3"], np.float32).T).astype(BF)
    aw = np.asarray(inp["attn_w"], np.float32)[0]
    watA_h = np.zeros((128, 4), np.float32)
    watA_h[0:64, 0] = aw
    watA_h[64:128, 1] = aw
    watB_h = np.zeros((128, 4), np.float32)
    watB_h[0:64, 2] = aw
    watB_h[64:128, 3] = aw
    brA_h = np.zeros((4, 128), np.float32)
    brA_h[0, 0:64] = 1
    brA_h[1, 64:128] = 1
    brB_h = np.zeros((4, 128), np.float32)
    brB_h[2, 0:64] = 1
    brB_h[3, 64:128] = 1
    ab4_h = np.full((4, 1), np.float32(np.asarray(inp["attn_b"]).reshape(-1)[0]))

    gbs = {1: ("pg1", "pbt1"), 2: ("pg2", "pbt2"), 3: ("pg3", "pbt3"),
           4: ("vg1", "vbt1"), 5: ("vg2", "vbt2"), 6: ("vg3", "vbt3")}
    gb_h = {k: np.stack([np.asarray(inp[g], np.float32),
                         np.asarray(inp[b], np.float32)], -1)
            for k, (g, b) in gbs.items()}

    ls32_h = np.zeros((128, 32), np.float32)
    for lane in range(4):
        ls32_h[32 * lane + np.arange(32), np.arange(32)] = 1
    ls48_h = np.zeros((96, 48), np.float32)
    for a in range(2):
        ls48_h[48 * a + np.arange(48), np.arange(48)] = 1
    ls64_h = np.zeros((128, 64), np.float32)
    for a in range(2):
        ls64_h[64 * a + np.arange(64), np.arange(64)] = 1
    lr32_h = np.zeros((32, 128), np.float32)
    for lane in range(4):
        lr32_h[np.arange(32), 32 * lane + np.arange(32)] = 1
    lr48_h = np.zeros((48, 96), np.float32)
    for a in range(2):
        lr48_h[np.arange(48), 48 * a + np.arange(48)] = 1
    lr64_h = np.zeros((64, 128), np.float32)
    for a in range(2):
        lr64_h[np.arange(64), 64 * a + np.arange(64)] = 1

    vw1_h = np.asarray(inp["vw1"], np.float32).transpose(1, 2, 3, 4, 0).reshape(64, 27 * 96).astype(BF)
    vw2_h = np.asarray(inp["vw2"], np.float32).transpose(1, 2, 0, 3, 4).reshape(96, 5 * 128).astype(BF)
    vw3_h = np.asarray(inp["vw3"], np.float32).transpose(1, 2, 0, 3, 4).reshape(128, 3 * 64).astype(BF)

    wih = np.asarray(inp["lstm_wih"], np.float32)
    whh = np.asarray(inp["lstm_whh"], np.float32)
    wihT_h = wih.transpose(0, 2, 1).transpose(1, 0, 2).reshape(64, 3 * 256).astype(np.float32)
    whhT_h = whh.transpose(0, 2, 1).transpose(1, 0, 2).reshape(64, 3 * 256).astype(np.float32)
    bih = np.asarray(inp["lstm_bih"], np.float32)
    bhh = np.asarray(inp["lstm_bhh"], np.float32)
    lb12_h = np.zeros((64, 24), np.float32)
    for l in range(3):
        for g in range(4):
            lb12_h[:, 4 * l + g] = bih[l, 64 * g:64 * (g + 1)]
            lb12_h[:, 12 + 4 * l + g] = bhh[l, 64 * g:64 * (g + 1)]
    h0T_h = np.asarray(inp["h0"], np.float32).transpose(0, 2, 1).transpose(1, 0, 2).reshape(64, 3 * Bg)
    c0T_h = np.asarray(inp["c0"], np.float32).transpose(0, 2, 1).transpose(1, 0, 2).reshape(64, 3 * Bg)

    lastT = cfg.TG - 1
    nlanes = NT - 4 * lastT

    def padmask(chw, lanes_in):
        pm = np.zeros((len(lanes_in) * chw, 1024), np.float32)
        for li, lane in enumerate(lanes_in):
            t = 4 * lastT + lane
            if t >= NT:
                continue
            for prow in range(128):
                if 128 * t + prow < R:
                    pm[chw * li:chw * (li + 1), 8 * prow:8 * (prow + 1)] = 1
        return pm.astype(BF)

    pm48a_h = padmask(48, [0, 1])
    pm48b_h = padmask(48, [2, 3])

    foff_h = np.zeros((128, NT), np.uint32)
    rr = (128 * np.arange(NT)[None, :] + np.arange(128)[:, None])
    valid = rr < R
    foff_h[valid] = (rr[valid] // S).astype(np.uint32) * N

    t4_h = np.zeros((4, GAP * FPC + 128), np.float32)
    cols = (GAP * np.arange(FPC)[:, None] + np.arange(S)[None, :]).ravel()
    for r in range(3):
        t4_h[r, cols] = np.tile(2.0 * tpl[:, r], FPC)
    t4_h[3, cols] = -1.0

    wsel_h = np.zeros((3 * VL, VL), np.float32)
    for v in range(VL):
        wsel_h[3 * v + np.arange(3), v] = 1

    in_maps = []
    for c in range(cfg.cores):
        xc = x[c * FPC:(c + 1) * FPC]  # [FPC, N, FEAT]
        xt_h = np.zeros((FPC * N, 32), BF)
        xt_h[:, :FEAT] = xc.reshape(FPC * N, FEAT).astype(BF)
        xyz = xc[:, :, 0:3].reshape(FB, VL, N, 3)
        xyzr_h = np.zeros((3 * VL, FB * N), np.float32)
        for r in range(3):
            m = xyz[:, :, :, r].transpose(1, 0, 2).reshape(VL, FB * N)
            xyzr_h[3 * np.arange(VL) + r] = m
        a4 = t4_h.copy()
        gseg = g_loc[c * FPC:(c + 1) * FPC]
        for r in range(2):
            a4[r, cols] += np.repeat(2.0 * gseg[:, r], S)
        a2dw_h = a4.astype(np.float32)
        xyzdr_h = np.ascontiguousarray(
            xc[:, :, 0:3].reshape(-1, 3).T.astype(np.float32))
        in_maps.append({
            "xt": xt_h, "xyzdr": xyzdr_h, "xyzr": xyzr_h, "a2dw": a2dw_h,
            "wsel": wsel_h, "wd3": wd3_h, "wtil": wtil_h,
            "w2": w2_h, "w3": w3_h, "watA": watA_h.astype(BF),
            "watB": watB_h.astype(BF), "brA": brA_h.astype(BF),
            "brB": brB_h.astype(BF), "ab4": ab4_h,
            **{f"gb{k}": v for k, v in gb_h.items()},
            "ls32": ls32_h, "ls48": ls48_h, "ls64": ls64_h,
            "lr32": lr32_h, "lr48": lr48_h, "lr64": lr64_h,
            "vw1": vw1_h, "vw2": vw2_h, "vw3": vw3_h,
            "wihT": wihT_h, "whhT": whhT_h, "lb12": lb12_h,
            "h0T": h0T_h, "c0T": c0T_h, "foff": foff_h,
            "pm48a": pm48a_h, "pm48b": pm48b_h,
            "identb": np.eye(128, dtype=np.float32).astype(BF),
        })
    return in_maps


def postprocess(cfg: Cfg, results):
    R, NT, TG, L, Bg = cfg.R, cfg.NT, cfg.TG, cfg.L, cfg.B
    attn_full = np.zeros((cfg.cores * R, NS), np.float32)
    for c, res in enumerate(results):
        arr = np.asarray(res["attn_o"]).astype(np.float32).reshape(4, TG, 128, NS)
        for t in range(NT):
            lane, T = t % 4, t // 4
            r0 = 128 * t
            n = min(128, R - r0)
            attn_full[c * R + r0:c * R + r0 + n] = arr[lane, T, :n]
    res0 = results[0]
    avec = np.asarray(res0["avec_o"]).reshape(64, L, Bg)
    a_vec = avec.transpose(2, 1, 0).astype(np.float32)
    hncn = np.asarray(res0["hncn_o"]).reshape(64, 6, Bg)
    hn = hncn[:, 0:3].transpose(1, 2, 0).astype(np.float32)
    cn = hncn[:, 3:6].transpose(1, 2, 0).astype(np.float32)
    return (a_vec, attn_full[:, :, None].astype(np.float32), hn, cn)


_CACHE = {}


def kernel(**inputs):
    cfg = Cfg()
    if "nc" not in _CACHE:
        _CACHE["nc"] = build_program(cfg)
    nc = _CACHE["nc"]
    in_maps = prep_inputs(cfg, inputs)
    from concourse import bass_utils
    res = bass_utils.run_bass_kernel_spmd(nc, in_maps, core_ids=list(range(cfg.cores)))
    return postprocess(cfg, res.results)
